# revision 7
# baseline (speedup 1.0000x reference)
"""GQA with sliding-window (W=512) sparse attention on 8 trn2 NeuronCores.

Sharding: core = b*4 + grp  (b in {0,1} batch, grp in {0..3} KV-head group).
Each core computes, for its batch b and its 4 query heads (one KV head):
  qT/kT/v projections, RoPE, banded softmax(QK^T) in [j,i] orientation,
  AV, and a partial x@Wo.T contribution (summed across the 4 groups on host).

Device outputs per core:
  band[h, g, r, c] = att[b, grp*4+h, i=g*128+c, j=g*128+r]  (bf16, staircase-masked)
  yp[t, c]         = partial y for batch b (f32)
Host assembles full (y, att); att outside the band is exactly zero.
"""

import sys

for _p in ("/opt/trn_rl_repo",):
    if _p not in sys.path:
        sys.path.insert(0, _p)

import numpy as np
import ml_dtypes

B, T, C = 2, 2048, 2048
D = 128          # head dim
W = 512          # sliding window
NH = 16          # total q heads
NKV = 4          # kv heads
GH = 4           # q heads per core (= NH // NKV)
NT = T // 128    # 16 tiles of 128
BW = 640         # band width per j-tile: 128 + W
ROPE_BASE = 10000.0

_bf16 = ml_dtypes.bfloat16
_COMPILED = {}


def _legalize_waits(nc, mybir):
    """TRN2 walrus codegen accepts at most one sync-wait per instruction.
    Hoist extra waits onto standalone EventSemaphore instructions inserted
    just before the over-subscribed instruction (same engine => same stream
    order, identical semantics)."""
    cnt = 0
    for f in nc.m.functions:
        for blk in f.blocks:
            out = []
            for ins in blk.instructions:
                si = ins.sync_info
                if si is not None and si.on_wait and len(si.on_wait) > 1:
                    waits = list(si.on_wait)
                    for w in waits[:-1]:
                        cnt += 1
                        out.append(
                            mybir.InstEventSemaphore(
                                name=f"EVW-{cnt}",
                                engine=ins.engine,
                                sync_info=mybir.SyncInfo(on_wait=[w], on_update=[]),
                            )
                        )
                    ins.sync_info = mybir.SyncInfo(
                        on_wait=[waits[-1]], on_update=list(si.on_update)
                    )
                out.append(ins)
            blk.instructions = out


def _build_nc():
    import concourse.bass as bass
    import concourse.mybir as mybir
    import concourse.tile as tile
    from contextlib import ExitStack

    f32 = mybir.dt.float32
    bf16 = mybir.dt.bfloat16
    AF = mybir.ActivationFunctionType
    ts = bass.ts

    nc = bass.Bass()

    xT = nc.declare_dram_parameter("xT", [C, T], bf16, isOutput=False)
    wqT = nc.declare_dram_parameter("wqT", [C, GH * D], bf16, isOutput=False)
    wkT = nc.declare_dram_parameter("wkT", [C, D], bf16, isOutput=False)
    wvT = nc.declare_dram_parameter("wvT", [C, D], bf16, isOutput=False)
    woT = nc.declare_dram_parameter("woT", [GH * D, C], bf16, isOutput=False)
    cosT = nc.declare_dram_parameter("cosT", [D // 2, T], f32, isOutput=False)
    sinT = nc.declare_dram_parameter("sinT", [D // 2, T], f32, isOutput=False)
    mask = nc.declare_dram_parameter("mask", [128, BW], bf16, isOutput=False)
    band = nc.declare_dram_parameter("band", [GH, NT, 128, BW], bf16, isOutput=True)
    yp = nc.declare_dram_parameter("yp", [T, C], f32, isOutput=True)

    with tile.TileContext(nc) as tc, ExitStack() as top:
        const = top.enter_context(tc.tile_pool(name="const", bufs=1))

        # Resident weights / tables
        wq_sb = const.tile([128, C // 128, GH * D], bf16, name="wq_sb")
        nc.sync.dma_start(wq_sb[:], wqT.rearrange("(n p) m -> p n m", p=128))
        wk_sb = const.tile([128, C // 128, D], bf16, name="wk_sb")
        nc.sync.dma_start(wk_sb[:], wkT.rearrange("(n p) m -> p n m", p=128))
        wv_sb = const.tile([128, C // 128, D], bf16, name="wv_sb")
        nc.sync.dma_start(wv_sb[:], wvT.rearrange("(n p) m -> p n m", p=128))
        wo_sb = const.tile([128, GH, C], bf16, name="wo_sb")
        nc.sync.dma_start(wo_sb[:], woT.rearrange("(h p) c -> p h c", p=128))
        cos_sb = const.tile([D // 2, T], f32, name="cos_sb")
        nc.sync.dma_start(cos_sb[:], cosT[:])
        sin_sb = const.tile([D // 2, T], f32, name="sin_sb")
        nc.sync.dma_start(sin_sb[:], sinT[:])
        mask_sb = const.tile([128, BW], bf16, name="mask_sb")
        nc.sync.dma_start(mask_sb[:], mask[:])
        ones_sb = const.tile([128, 1], bf16, name="ones_sb")
        nc.gpsimd.memset(ones_sb[:], 1.0)
        onesf_sb = const.tile([1, 128], f32, name="onesf_sb")
        nc.gpsimd.memset(onesf_sb[:], 1.0)

        # Resident activations
        q_sb = const.tile([128, GH, T], bf16, name="q_sb")   # [d, h, t] roped
        k_sb = const.tile([128, T], bf16, name="k_sb")       # [d, j] roped+scaled
        v_sb = const.tile([128, NT, D], bf16, name="v_sb")   # [j%128, jtile, d]
        yT_sb = const.tile([128, GH, T], bf16, name="yT_sb")  # [d, h, t] normalized

        # ---------------- Phase 1: projections + RoPE ----------------
        with ExitStack() as ctx:
            xpool = ctx.enter_context(tc.tile_pool(name="xpool", bufs=2))
            rtmp = ctx.enter_context(tc.tile_pool(name="rtmp", bufs=4))
            ps1 = ctx.enter_context(
                tc.tile_pool(name="ps1", bufs=2, space="PSUM")
            )

            def rope_drain(ps, out_top, out_bot, nsl):
                cs = cos_sb[:, nsl]
                sn = sin_sb[:, nsl]
                t1 = rtmp.tile([64, 512], f32, name="t1", tag="t1")
                t2 = rtmp.tile([64, 512], f32, name="t2", tag="t2")
                nc.vector.tensor_mul(t1[:], ps[0:64, :], cs)
                nc.vector.tensor_mul(t2[:], ps[64:128, :], sn)
                nc.vector.tensor_sub(out_top, t1[:], t2[:])
                t3 = rtmp.tile([64, 512], f32, name="t3", tag="t1")
                t4 = rtmp.tile([64, 512], f32, name="t4", tag="t2")
                nc.vector.tensor_mul(t3[:], ps[64:128, :], cs)
                nc.vector.tensor_mul(t4[:], ps[0:64, :], sn)
                nc.vector.tensor_add(out_bot, t3[:], t4[:])

            for n in range(4):  # t-slices of 512
                nsl = slice(n * 512, (n + 1) * 512)
                x_sb = xpool.tile([128, C // 128, 512], bf16, name="x_sb", tag="x")
                nc.sync.dma_start(
                    x_sb[:], xT.rearrange("(n p) t -> p n t", p=128)[:, :, nsl]
                )
                # q heads + k
                for m in range(GH + 1):
                    ps = ps1.tile([128, 512], f32, name="proj_ps", tag="proj")
                    for cc in range(C // 128):
                        lhsT = (
                            wq_sb[:, cc, ts(m, 128)]
                            if m < GH
                            else wk_sb[:, cc, :]
                        )
                        nc.tensor.matmul(
                            ps[:],
                            lhsT,
                            x_sb[:, cc, :],
                            start=(cc == 0),
                            stop=(cc == C // 128 - 1),
                        )
                    if m < GH:
                        rope_drain(
                            ps[:], q_sb[0:64, m, nsl], q_sb[64:128, m, nsl], nsl
                        )
                    else:
                        rope_drain(
                            ps[:], k_sb[0:64, nsl], k_sb[64:128, nsl], nsl
                        )
                # v: [t, d] tiles — one psum tile per group (a start=True
                # reset clears the whole bank, so groups must not share one)
                for tm in range(4):
                    vps = ps1.tile([128, D], f32, name="v_ps", tag="vps", bufs=2)
                    for cc in range(C // 128):
                        nc.tensor.matmul(
                            vps[:],
                            x_sb[:, cc, ts(tm, 128)],
                            wv_sb[:, cc, :],
                            start=(cc == 0),
                            stop=(cc == C // 128 - 1),
                        )
                    nc.scalar.copy(v_sb[:, n * 4 + tm, :], vps[:])

        # ---------------- Phase 2: banded attention ----------------
        with ExitStack() as ctx:
            ppool = ctx.enter_context(tc.tile_pool(name="ppool", bufs=1))
            rpool = ctx.enter_context(tc.tile_pool(name="rpool", bufs=2))
            bpool = ctx.enter_context(tc.tile_pool(name="bpool", bufs=3))
            ytmp = ctx.enter_context(tc.tile_pool(name="ytmp", bufs=2))
            ps2 = ctx.enter_context(
                tc.tile_pool(name="ps2", bufs=2, space="PSUM")
            )

            for h in range(GH):
                p_all = ppool.tile([128, NT, BW], bf16, name="p_all", tag="pall")
                # B1: scores -> exp -> mask
                for g in range(NT):
                    w = min(BW, T - g * 128)
                    sps = ps2.tile([128, BW], f32, name="s_ps", tag="s", bufs=2)
                    w0 = min(512, w)
                    nc.tensor.matmul(
                        sps[:, 0:w0],
                        k_sb[:, ts(g, 128)],
                        q_sb[:, h, g * 128 : g * 128 + w0],
                        start=True,
                        stop=True,
                    )
                    if w > 512:
                        nc.tensor.matmul(
                            sps[:, 512:w],
                            k_sb[:, ts(g, 128)],
                            q_sb[:, h, g * 128 + 512 : g * 128 + w],
                            start=True,
                            stop=True,
                        )
                    nc.scalar.activation(
                        p_all[:, g, 0:w], sps[:, 0:w], mybir.ActivationFunctionType.Exp
                    )
                    nc.gpsimd.tensor_mul(
                        p_all[:, g, 0:w], p_all[:, g, 0:w], mask_sb[:, 0:w]
                    )
                # B2: row sums -> 1/sum
                recip_row = rpool.tile([1, T], f32, name="recip_row", tag="rrow")
                for ii in range(NT):
                    g0 = max(0, ii - 4)
                    rs = ps2.tile([1, 128], f32, name="rs_ps", tag="rs", bufs=1)
                    for g in range(g0, ii + 1):
                        c0 = (ii - g) * 128
                        nc.tensor.matmul(
                            rs[:],
                            ones_sb[:],
                            p_all[:, g, c0 : c0 + 128],
                            start=(g == g0),
                            stop=(g == ii),
                        )
                    nc.vector.reciprocal(recip_row[0:1, ts(ii, 128)], rs[:])
                recip_bc = rpool.tile([128, T], bf16, name="recip_bc", tag="rbc")
                for q4 in range(4):
                    bc = ps2.tile([128, 512], f32, name="bc_ps", tag="bc", bufs=1)
                    nc.tensor.matmul(
                        bc[:],
                        onesf_sb[:],
                        recip_row[0:1, ts(q4, 512)],
                        start=True,
                        stop=True,
                    )
                    nc.scalar.copy(recip_bc[:, ts(q4, 512)], bc[:])
                # B3: AV + normalize
                for ii in range(NT):
                    g0 = max(0, ii - 4)
                    yt = ps2.tile([128, 128], f32, name="yt_ps", tag="yt", bufs=2)
                    for g in range(g0, ii + 1):
                        c0 = (ii - g) * 128
                        nc.tensor.matmul(
                            yt[:],
                            v_sb[:, g, :],
                            p_all[:, g, c0 : c0 + 128],
                            start=(g == g0),
                            stop=(g == ii),
                        )
                    ytt = ytmp.tile([128, 128], bf16, name="ytt", tag="ytt")
                    nc.scalar.copy(ytt[:], yt[:])
                    nc.vector.tensor_mul(
                        yT_sb[:, h, ts(ii, 128)], ytt[:], recip_bc[:, ts(ii, 128)]
                    )
                # band output: normalize p and store
                for g in range(NT):
                    w = min(BW, T - g * 128)
                    bs = bpool.tile([128, BW], bf16, name="bs", tag="bs")
                    nc.vector.tensor_mul(
                        bs[:, 0:w],
                        p_all[:, g, 0:w],
                        recip_bc[:, g * 128 : g * 128 + w],
                    )
                    nc.sync.dma_start(band[h, g, :, 0:w], bs[:, 0:w])

        # ---------------- Phase 3: output projection ----------------
        with ExitStack() as ctx:
            opool = ctx.enter_context(tc.tile_pool(name="opool", bufs=3))
            ps3 = ctx.enter_context(
                tc.tile_pool(name="ps3", bufs=2, space="PSUM")
            )
            for n4 in range(4):
                for mt in range(NT):
                    ops = ps3.tile([128, 512], f32, name="o_ps", tag="o")
                    for h in range(GH):
                        nc.tensor.matmul(
                            ops[:],
                            yT_sb[:, h, ts(mt, 128)],
                            wo_sb[:, h, ts(n4, 512)],
                            start=(h == 0),
                            stop=(h == GH - 1),
                        )
                    ost = opool.tile([128, 512], f32, name="ost", tag="ost")
                    nc.scalar.copy(ost[:], ops[:])
                    nc.sync.dma_start(
                        yp[ts(mt, 128), ts(n4, 512)], ost[:]
                    )

    _legalize_waits(nc, mybir)
    return nc


def _get_nc():
    if "nc" not in _COMPILED:
        _COMPILED["nc"] = _build_nc()
    return _COMPILED["nc"]


def _host_tables():
    if "tabs" in _COMPILED:
        return _COMPILED["tabs"]
    inv_freq = 1.0 / (ROPE_BASE ** (np.arange(0, D, 2, dtype=np.float64) / D))
    t = np.arange(T, dtype=np.float64)
    freqs = np.outer(t, inv_freq)  # (T, 64)
    cosT = np.cos(freqs).T.astype(np.float32).copy()  # (64, T)
    sinT = np.sin(freqs).T.astype(np.float32).copy()
    r = np.arange(128)[:, None]
    c = np.arange(BW)[None, :]
    mask = ((c >= r) & (c - r < W)).astype(_bf16)  # (128, 640)
    _COMPILED["tabs"] = (cosT, sinT, mask)
    return _COMPILED["tabs"]


def kernel(x, attn_mask, Wq, Wk, Wv, Wo):
    from concourse.bass_utils import run_bass_kernel_spmd

    nc = _get_nc()
    cosT, sinT, mask = _host_tables()

    x = np.asarray(x, dtype=np.float32)
    Wq = np.asarray(Wq, dtype=np.float32)
    Wk = np.asarray(Wk, dtype=np.float32)
    Wv = np.asarray(Wv, dtype=np.float32)
    Wo = np.asarray(Wo, dtype=np.float32)

    in_maps = []
    for core in range(8):
        b, grp = divmod(core, 4)
        in_maps.append(
            {
                "xT": np.ascontiguousarray(x[b].T).astype(_bf16),
                "wqT": np.ascontiguousarray(
                    Wq[grp * GH * D : (grp + 1) * GH * D, :].T
                ).astype(_bf16),
                "wkT": np.ascontiguousarray(
                    Wk[grp * D : (grp + 1) * D, :].T / np.sqrt(D)
                ).astype(_bf16),
                "wvT": np.ascontiguousarray(
                    Wv[grp * D : (grp + 1) * D, :].T
                ).astype(_bf16),
                "woT": np.ascontiguousarray(
                    Wo[:, grp * GH * D : (grp + 1) * GH * D].T
                ).astype(_bf16),
                "cosT": cosT,
                "sinT": sinT,
                "mask": mask,
            }
        )

    res = run_bass_kernel_spmd(nc, in_maps, list(range(8)))

    y = np.zeros((B, T, C), dtype=np.float32)
    att = np.zeros((B, NH, T, T), dtype=np.float32)
    for core in range(8):
        b, grp = divmod(core, 4)
        out = res.results[core]
        y[b] += out["yp"]
        bandf = np.asarray(out["band"], dtype=np.float32)  # (GH, NT, 128, BW)
        for h in range(GH):
            H = grp * GH + h
            for g in range(NT):
                w = min(BW, T - g * 128)
                att[b, H, g * 128 : g * 128 + w, g * 128 : g * 128 + 128] = bandf[
                    h, g, :, 0:w
                ].T
    return (y, att)


if __name__ == "__main__":
    rng = np.random.default_rng(0)
    ins = {
        "x": rng.standard_normal((B, T, C), dtype=np.float32),
        "attn_mask": np.ones((1, 1, T, T), np.float32),
        "Wq": (rng.standard_normal((C, C), dtype=np.float32) * 0.02),
        "Wk": (rng.standard_normal((NKV * D, C), dtype=np.float32) * 0.02),
        "Wv": (rng.standard_normal((NKV * D, C), dtype=np.float32) * 0.02),
        "Wo": (rng.standard_normal((C, C), dtype=np.float32) * 0.02),
    }
    y, att = kernel(**ins)
    print("ok", y.shape, att.shape, float(np.abs(y).mean()), float(att.sum()))
